# revision 1
# baseline (speedup 1.0000x reference)
"""Trainium2 Bass kernel for nn_DifferentiableParticleSystem (N=2048 GNN message passing).

Strategy (8 NeuronCores, shard receiver rows i):
  - Each core owns 256 receiver particles i; positions/velocities/mass replicated.
  - Edge features are generated in [i-partition, j-free] plane layout (full 128-lane
    DVE utilization), gathered per-i into [8, 2048] feature-major layout via DMA.
  - The 4-layer edge MLP runs on the PE as 2 block-diagonal fp16 matmuls per chunk:
      pass X = L1 (feat->h1) + L3 (h2->h3),  pass Y = L2 (h1->h2) + L4 (h3->f)
    with biases folded in via a constant-ones rhs row. 4 independent j-streams
    (jb = 0..3, 512 columns each) pipeline through PSUM.
  - relu(X) on ScalarE, relu(Y) on VectorE; raw z4 (pre-tanh) is scattered back to
    [i, j] planes; tanh, the (dist<1) mask (x10 folded in) and the sum over j run
    in plane layout. The diagonal (j==i) term is removed by subtracting
    f_diag = 10*tanh(MLP([0..0,1])), computed on-device with the same weights.
  - Particle integration epilogue runs per 128-i half in [i-partition, 3] layout.
"""

import numpy as np

N = 2048
NCORES = 8
NI_CORE = N // NCORES  # 256 receiver rows per core
DT_STEP = 0.016
GRAV_Y = -9.8

_RUNNER = None


# ----------------------------------------------------------------------------
# Bass program
# ----------------------------------------------------------------------------

def _build_nc(ni_core=NI_CORE):
    import concourse.bacc as bacc
    import concourse.mybir as mybir
    from concourse.tile import TileContext
    from concourse.mybir import AluOpType as op

    f32 = mybir.dt.float32
    f16 = mybir.dt.float16
    AF = mybir.ActivationFunctionType
    AX = mybir.AxisListType

    J = N              # sender dimension (full)
    S = 4              # j-streams
    FD = J // S        # 512 free columns per stream
    halves = [(h * 128, min(128, ni_core - h * 128))
              for h in range((ni_core + 127) // 128)]

    nc = bacc.Bacc("TRN2", target_bir_lowering=False)

    # ---- DRAM I/O ----
    posT = nc.dram_tensor("posT", [3, J], f32, kind="ExternalInput")
    velT = nc.dram_tensor("velT", [3, J], f32, kind="ExternalInput")
    massR = nc.dram_tensor("massR", [1, J], f32, kind="ExternalInput")
    pos_my = nc.dram_tensor("pos_my", [ni_core, 3], f32, kind="ExternalInput")
    vel_my = nc.dram_tensor("vel_my", [ni_core, 3], f32, kind="ExternalInput")
    mass_my = nc.dram_tensor("mass_my", [ni_core, 1], f32, kind="ExternalInput")
    ext_my = nc.dram_tensor("ext_my", [ni_core, 3], f32, kind="ExternalInput")
    ela_my = nc.dram_tensor("ela_my", [ni_core, 1], f32, kind="ExternalInput")
    fri_my = nc.dram_tensor("fri_my", [ni_core, 1], f32, kind="ExternalInput")
    W1d = nc.dram_tensor("W1", [8, 64], f32, kind="ExternalInput")
    b1d = nc.dram_tensor("b1", [1, 64], f32, kind="ExternalInput")
    W2d = nc.dram_tensor("W2", [64, 64], f32, kind="ExternalInput")
    b2d = nc.dram_tensor("b2", [1, 64], f32, kind="ExternalInput")
    W3d = nc.dram_tensor("W3", [64, 32], f32, kind="ExternalInput")
    b3d = nc.dram_tensor("b3", [1, 32], f32, kind="ExternalInput")
    W4d = nc.dram_tensor("W4", [32, 3], f32, kind="ExternalInput")
    b4d = nc.dram_tensor("b4", [1, 3], f32, kind="ExternalInput")
    out_pos = nc.dram_tensor("out_pos", [ni_core, 3], f32, kind="ExternalOutput")
    out_vel = nc.dram_tensor("out_vel", [ni_core, 3], f32, kind="ExternalOutput")

    # ---- persistent SBUF ----
    def sb(name, shape, dt):
        return nc.alloc_sbuf_tensor(name, shape, dt)

    pj = [sb(f"pj{d}", [128, J], f16) for d in range(3)]     # pos_j bcast planes
    vj = [sb(f"vj{d}", [128, J], f16) for d in range(3)]     # vel_j bcast planes
    imj = sb("imj", [128, J], f16)                           # 1/mass_j bcast
    FP = [sb(f"fp{h}", [128, 8 * J], f16) for h in range(len(halves))]
    ZP = [sb(f"zp{h}", [128, 3 * J], f16) for h in range(len(halves))]
    scrA = sb("scrA", [128, J], f16)
    scrB = sb("scrB", [128, J], f16)
    scr32 = sb("scr32", [128, J], f32)
    maskS = sb("maskS", [128, J], f16)
    lhsTX = sb("lhsTX", [104, 96], f16)      # W3_hi @ rows 0-63, W1_hi @ 96-103
    lhsTXl = sb("lhsTXl", [104, 96], f16)    # W3_lo / W1_lo residuals
    lhsTY = sb("lhsTY", [96, 67], f16)       # W2_hi @ rows 0-63, W4_hi @ 64-95
    lhsTYl = sb("lhsTYl", [96, 67], f16)     # W2_lo / W4_lo residuals
    RX = [sb(f"rx{r}", [104, J], f16) for r in range(3)]     # rhs ring for pass X
    relucol = sb("relucol", [67, 1], f32)                    # 0 / -inf max operand
    b13col = sb("b13col", [96, 1], f32)                      # [b1; b3] relu-X bias
    byc = sb("byc", [67, 1], f32)                            # [b2; b4] pass-Y bias
    RY = [sb(f"ry{r}", [96, J], f16) for r in range(2)]      # rhs ring for pass Y
    negfd = sb("negfd", [128, 3], f32)                       # -10*tanh(z4_diag)
    nf = [sb(f"nf{h}", [128, 3], f32) for h in range(len(halves))]

    Xps = nc.alloc_psum_tensor("xps", [96, J], f32)
    Yps = nc.alloc_psum_tensor("yps", [67, J], f32)

    with TileContext(nc) as tc:
        import contextlib
        with contextlib.ExitStack() as ctx:
            pool = ctx.enter_context(tc.tile_pool(name="misc", bufs=1))

            # ================= setup: weights (fp16 hi + lo residual) =================
            nc.vector.memset(lhsTX.ap()[:, :], 0.0)
            nc.vector.memset(lhsTXl.ap()[:, :], 0.0)
            nc.vector.memset(lhsTY.ap()[:, :], 0.0)
            nc.vector.memset(lhsTYl.ap()[:, :], 0.0)

            def load_wpair(dram_ap, rows, cols, hi_t, lo_t, tagn):
                nr, ncol = rows.stop - rows.start, cols.stop - cols.start
                w32 = pool.tile([64, 64], f32, tag="w32s")
                nc.sync.dma_start(w32[0:nr, 0:ncol], dram_ap)
                hi16 = pool.tile([64, 64], f16, tag="w16h")
                nc.vector.tensor_copy(hi16[0:nr, 0:ncol], w32[0:nr, 0:ncol])
                nc.vector.tensor_copy(hi_t.ap()[rows, cols], hi16[0:nr, 0:ncol])
                res = pool.tile([64, 64], f32, tag="w32r")
                nc.vector.tensor_tensor(res[0:nr, 0:ncol], w32[0:nr, 0:ncol],
                                        hi16[0:nr, 0:ncol], op.subtract)
                nc.vector.tensor_copy(lo_t.ap()[rows, cols], res[0:nr, 0:ncol])

            load_wpair(W1d[:, :], slice(96, 104), slice(0, 64), lhsTX, lhsTXl, "w1")
            load_wpair(W3d[:, :], slice(0, 64), slice(64, 96), lhsTX, lhsTXl, "w3")
            load_wpair(W2d[:, :], slice(0, 64), slice(0, 64), lhsTY, lhsTYl, "w2")
            load_wpair(W4d[:, :], slice(64, 96), slice(64, 67), lhsTY, lhsTYl, "w4")
            # bias columns ([1,n] dram rows -> [n,1] sbuf cols via swdge dma)
            nc.gpsimd.dma_start(b13col.ap()[0:64, 0:1], b1d[0:1, 0:64])
            nc.gpsimd.dma_start(b13col.ap()[64:96, 0:1], b3d[0:1, 0:32])
            nc.gpsimd.dma_start(byc.ap()[0:64, 0:1], b2d[0:1, 0:64])
            nc.gpsimd.dma_start(byc.ap()[64:67, 0:1], b4d[0:1, 0:3])

            # ================= setup: broadcast planes =================
            for d in range(3):
                r32 = pool.tile([1, J], f32, tag="rowstg32")
                r16 = pool.tile([1, J], f16, tag="rowstg16")
                nc.sync.dma_start(r32[:, :], posT[d:d + 1, :])
                nc.vector.tensor_copy(r16[:, :], r32[:, :])
                nc.gpsimd.partition_broadcast(pj[d].ap()[:, :], r16[0:1, :])
                r32v = pool.tile([1, J], f32, tag="rowstg32")
                r16v = pool.tile([1, J], f16, tag="rowstg16")
                nc.sync.dma_start(r32v[:, :], velT[d:d + 1, :])
                nc.vector.tensor_copy(r16v[:, :], r32v[:, :])
                nc.gpsimd.partition_broadcast(vj[d].ap()[:, :], r16v[0:1, :])
            stgm = pool.tile([1, J], f32, tag="rowstg32")
            stgm16 = pool.tile([1, J], f16, tag="rowstg16")
            nc.sync.dma_start(stgm[:, :], massR[:, :])
            nc.vector.reciprocal(stgm[:, :], stgm[:, :])
            nc.vector.tensor_copy(stgm16[:, :], stgm[:, :])
            nc.gpsimd.partition_broadcast(imj.ap()[:, :], stgm16[0:1, :])

            # ================= setup: ring constants =================
            for r in range(3):
                nc.gpsimd.memset(RX[r].ap()[64:96, :], 0.0)
            nc.gpsimd.memset(RX[0].ap()[0:64, :], 0.0)
            nc.gpsimd.memset(relucol.ap()[0:64, :], 0.0)
            nc.gpsimd.memset(relucol.ap()[64:67, :], -1e30)

            # ================= feature planes =================
            picol = {}
            for h, (i0, Ph) in enumerate(halves):
                for d in range(3):
                    pc = pool.tile([128, 1], f32, tag=f"pic{h}{d}")
                    nc.sync.dma_start(pc[0:Ph, :], pos_my[i0:i0 + Ph, d:d + 1])
                    picol[(h, d)] = pc
                    nc.vector.tensor_scalar(
                        FP[h].ap()[0:Ph, d * J:(d + 1) * J], pj[d].ap()[0:Ph, :],
                        pc[0:Ph, :], None, op.subtract)
                    vc = pool.tile([128, 1], f32, tag=f"vic{h}{d}")
                    nc.sync.dma_start(vc[0:Ph, :], vel_my[i0:i0 + Ph, d:d + 1])
                    nc.vector.tensor_scalar(
                        FP[h].ap()[0:Ph, (3 + d) * J:(4 + d) * J], vj[d].ap()[0:Ph, :],
                        vc[0:Ph, :], None, op.subtract)
                # dist = sqrt(dx^2+dy^2+dz^2)
                rel = lambda d: FP[h].ap()[0:Ph, d * J:(d + 1) * J]
                nc.vector.tensor_tensor(scrA.ap()[0:Ph, :], rel(0), rel(0), op.mult)
                nc.vector.tensor_tensor(scrB.ap()[0:Ph, :], rel(1), rel(1), op.mult)
                nc.vector.tensor_tensor(scrA.ap()[0:Ph, :], scrA.ap()[0:Ph, :],
                                        scrB.ap()[0:Ph, :], op.add)
                nc.vector.tensor_tensor(scrB.ap()[0:Ph, :], rel(2), rel(2), op.mult)
                nc.vector.tensor_tensor(scrA.ap()[0:Ph, :], scrA.ap()[0:Ph, :],
                                        scrB.ap()[0:Ph, :], op.add)
                nc.scalar.activation(FP[h].ap()[0:Ph, 6 * J:7 * J],
                                     scrA.ap()[0:Ph, :], AF.Sqrt)
                # mass ratio
                mc = pool.tile([128, 1], f32, tag=f"mic{h}")
                nc.sync.dma_start(mc[0:Ph, :], mass_my[i0:i0 + Ph, :])
                nc.vector.tensor_scalar(FP[h].ap()[0:Ph, 7 * J:8 * J],
                                        imj.ap()[0:Ph, :], mc[0:Ph, :], None, op.mult)

            # ================= f_diag (on-device, same weights) =================
            def mm_x(out_ap, rhs_ap):
                nc.tensor.matmul(out_ap, lhsTX.ap()[:, :], rhs_ap, start=True,
                                 stop=False)
                nc.tensor.matmul(out_ap, lhsTXl.ap()[:, :], rhs_ap, start=False,
                                 stop=True)

            def mm_y(out_ap, rhs_ap):
                nc.tensor.matmul(out_ap, lhsTY.ap()[:, :], rhs_ap, start=True,
                                 stop=False)
                nc.tensor.matmul(out_ap, lhsTYl.ap()[:, :], rhs_ap, start=False,
                                 stop=True)

            fdx1 = pool.tile([104, 1], f16, tag="fdx1")
            nc.gpsimd.memset(fdx1[0:96, :], 0.0)
            r9a = pool.tile([1, 8], f16, tag="r9a")
            nc.gpsimd.memset(r9a[:, :], 0.0)
            nc.gpsimd.memset(r9a[0:1, 7:8], 1.0)  # ratio feature = 1
            nc.gpsimd.dma_start(fdx1[96:104, 0:1], r9a[0:1, 0:8])
            mm_x(Xps.ap()[0:96, 0:1], fdx1[0:104, :])
            fdy1 = pool.tile([96, 1], f16, tag="fdy1")
            nc.scalar.activation(fdy1[0:96, :], Xps.ap()[0:96, 0:1], AF.Relu,
                                 bias=b13col.ap()[0:96, :])
            mm_y(Yps.ap()[0:67, 0:1], fdy1[0:96, :])
            fdx2 = pool.tile([104, 1], f16, tag="fdx2")
            nc.gpsimd.memset(fdx2[64:104, :], 0.0)
            nc.vector.tensor_scalar(fdx2[0:64, :], Yps.ap()[0:64, 0:1],
                                    byc.ap()[0:64, :], relucol.ap()[0:64, :],
                                    op.add, op.max)
            mm_x(Xps.ap()[0:96, 0:1], fdx2[0:104, :])
            fdy2 = pool.tile([96, 1], f16, tag="fdy2")
            nc.scalar.activation(fdy2[0:96, :], Xps.ap()[0:96, 0:1], AF.Relu,
                                 bias=b13col.ap()[0:96, :])
            mm_y(Yps.ap()[0:67, 0:1], fdy2[0:96, :])
            fd3 = pool.tile([3, 1], f32, tag="fd3")
            nc.scalar.activation(fd3[:, :], Yps.ap()[64:67, 0:1], AF.Tanh,
                                 bias=byc.ap()[64:67, :])
            nc.vector.tensor_scalar(fd3[:, :], fd3[:, :], -10.0, None, op.mult)
            fdrow = pool.tile([1, 3], f32, tag="fdrow")
            nc.gpsimd.dma_start(fdrow[0:1, 0:3], fd3[0:3, 0:1])
            nc.gpsimd.partition_broadcast(negfd.ap()[:, :], fdrow[0:1, :])

            # ================= main pipeline =================
            def do_post(h):
                i0, Ph = halves[h]
                nc.vector.tensor_scalar(maskS.ap()[0:Ph, :],
                                        FP[h].ap()[0:Ph, 6 * J:7 * J],
                                        1.0, 10.0, op.is_lt, op.mult)
                for d in range(3):
                    nc.scalar.activation(scr32.ap()[0:Ph, :],
                                         ZP[h].ap()[0:Ph, d * J:(d + 1) * J], AF.Tanh)
                    nc.vector.tensor_tensor(scr32.ap()[0:Ph, :], scr32.ap()[0:Ph, :],
                                            maskS.ap()[0:Ph, :], op.mult)
                    nc.vector.reduce_sum(nf[h].ap()[0:Ph, d:d + 1],
                                         scr32.ap()[0:Ph, :], AX.X)
                nc.vector.tensor_tensor(nf[h].ap()[0:Ph, :], nf[h].ap()[0:Ph, :],
                                        negfd.ap()[0:Ph, :], op.add)
                # ---- particle update epilogue ----
                ext = pool.tile([128, 3], f32, tag=f"ext{h}")
                nc.sync.dma_start(ext[0:Ph, :], ext_my[i0:i0 + Ph, :])
                pcol = pool.tile([128, 3], f32, tag=f"pc3{h}")
                nc.sync.dma_start(pcol[0:Ph, :], pos_my[i0:i0 + Ph, :])
                vcol = pool.tile([128, 3], f32, tag=f"vc3{h}")
                nc.sync.dma_start(vcol[0:Ph, :], vel_my[i0:i0 + Ph, :])
                mcol = pool.tile([128, 1], f32, tag=f"mm{h}")
                nc.sync.dma_start(mcol[0:Ph, :], mass_my[i0:i0 + Ph, :])
                ecol = pool.tile([128, 1], f32, tag=f"ee{h}")
                nc.sync.dma_start(ecol[0:Ph, :], ela_my[i0:i0 + Ph, :])
                fcol = pool.tile([128, 1], f32, tag=f"ff{h}")
                nc.sync.dma_start(fcol[0:Ph, :], fri_my[i0:i0 + Ph, :])

                frc = pool.tile([128, 3], f32, tag=f"frc{h}")
                nc.vector.tensor_tensor(frc[0:Ph, :], nf[h].ap()[0:Ph, :],
                                        ext[0:Ph, :], op.add)
                nc.vector.scalar_tensor_tensor(frc[0:Ph, 1:2], mcol[0:Ph, :],
                                               GRAV_Y, frc[0:Ph, 1:2],
                                               op.mult, op.add)
                imc = pool.tile([128, 1], f32, tag=f"imc{h}")
                nc.vector.reciprocal(imc[0:Ph, :], mcol[0:Ph, :])
                nc.vector.tensor_scalar(imc[0:Ph, :], imc[0:Ph, :], DT_STEP, None,
                                        op.mult)
                v1 = pool.tile([128, 3], f32, tag=f"v1{h}")
                nc.vector.scalar_tensor_tensor(v1[0:Ph, :], frc[0:Ph, :],
                                               imc[0:Ph, :], vcol[0:Ph, :],
                                               op.mult, op.add)
                sq = pool.tile([128, 3], f32, tag=f"sq{h}")
                nc.vector.tensor_tensor(sq[0:Ph, :], v1[0:Ph, :], v1[0:Ph, :],
                                        op.mult)
                spd = pool.tile([128, 1], f32, tag=f"spd{h}")
                nc.vector.reduce_sum(spd[0:Ph, :], sq[0:Ph, :], AX.X)
                nc.scalar.activation(spd[0:Ph, :], spd[0:Ph, :], AF.Sqrt)
                fm = pool.tile([128, 1], f32, tag=f"fm{h}")
                nc.vector.tensor_scalar(fm[0:Ph, :], spd[0:Ph, :], 0.1, None,
                                        op.is_gt)
                ffac = pool.tile([128, 1], f32, tag=f"ffac{h}")
                nc.vector.tensor_tensor(ffac[0:Ph, :], fcol[0:Ph, :], fm[0:Ph, :],
                                        op.mult)
                nc.vector.tensor_scalar(ffac[0:Ph, :], ffac[0:Ph, :], -DT_STEP, 1.0,
                                        op.mult, op.add)
                nc.vector.tensor_scalar(v1[0:Ph, :], v1[0:Ph, :], ffac[0:Ph, :],
                                        None, op.mult)
                p2 = pool.tile([128, 3], f32, tag=f"p2{h}")
                nc.vector.scalar_tensor_tensor(p2[0:Ph, :], v1[0:Ph, :], DT_STEP,
                                               pcol[0:Ph, :], op.mult, op.add)
                cm = pool.tile([128, 1], f32, tag=f"cm{h}")
                nc.vector.tensor_scalar(cm[0:Ph, :], p2[0:Ph, 1:2], 0.0, None,
                                        op.is_lt)
                ncm = pool.tile([128, 1], f32, tag=f"ncm{h}")
                nc.vector.tensor_scalar(ncm[0:Ph, :], cm[0:Ph, :], -1.0, 1.0,
                                        op.mult, op.add)
                nc.vector.tensor_tensor(p2[0:Ph, 1:2], p2[0:Ph, 1:2], ncm[0:Ph, :],
                                        op.mult)
                w1e = pool.tile([128, 1], f32, tag=f"w1e{h}")
                nc.vector.tensor_scalar(w1e[0:Ph, :], ecol[0:Ph, :], 1.0, None,
                                        op.add)
                nc.vector.tensor_tensor(w1e[0:Ph, :], cm[0:Ph, :], w1e[0:Ph, :],
                                        op.mult)
                nc.vector.tensor_scalar(w1e[0:Ph, :], w1e[0:Ph, :], -1.0, 1.0,
                                        op.mult, op.add)
                nc.vector.tensor_tensor(v1[0:Ph, 1:2], v1[0:Ph, 1:2], w1e[0:Ph, :],
                                        op.mult)
                nc.sync.dma_start(out_pos[i0:i0 + Ph, :], p2[0:Ph, :])
                nc.sync.dma_start(out_vel[i0:i0 + Ph, :], v1[0:Ph, :])

            for k in range(ni_core + 1):
                rx = RX[k % 3].ap()
                ry = RY[k % 2].ap()
                if k < ni_core:
                    h, p = (0, k) if k < 128 else (1, k - 128)
                    src = FP[h].ap()[p:p + 1, :].rearrange("p (d j) -> p d j", d=8)
                    nc.gpsimd.dma_start(rx[96:104, :], src)
                for lhs, st in ((lhsTX, True), (lhsTXl, False)):
                    for r in range(S):
                        sl = slice(r * FD, (r + 1) * FD)
                        nc.tensor.matmul(Xps.ap()[0:96, sl], lhs.ap()[:, :],
                                         rx[0:104, sl], start=st, stop=not st)
                for r in range(S):
                    sl = slice(r * FD, (r + 1) * FD)
                    nc.scalar.activation(ry[0:96, sl], Xps.ap()[0:96, sl], AF.Relu,
                                         bias=b13col.ap()[0:96, :])
                for lhs, st in ((lhsTY, True), (lhsTYl, False)):
                    for r in range(S):
                        sl = slice(r * FD, (r + 1) * FD)
                        nc.tensor.matmul(Yps.ap()[0:67, sl], lhs.ap()[:, :],
                                         ry[0:96, sl], start=st, stop=not st)
                rxn = RX[(k + 1) % 3].ap()
                for r in range(S):
                    sl = slice(r * FD, (r + 1) * FD)
                    nc.vector.tensor_scalar(rxn[0:67, sl], Yps.ap()[0:67, sl],
                                            byc.ap()[0:67, :], relucol.ap()[0:67, :],
                                            op.add, op.max)
                if k > 0:
                    i = k - 1
                    h, p = (0, i) if i < 128 else (1, i - 128)
                    dst = ZP[h].ap()[p:p + 1, :].rearrange("p (d j) -> p d j", d=3)
                    nc.sync.dma_start(dst, rxn[64:67, :])
                if k == 128 and len(halves) > 1:
                    do_post(0)
            do_post(len(halves) - 1)

    nc.compile()
    return nc


# ----------------------------------------------------------------------------
# Host-side runner (persistent jitted executable, modeled on
# concourse.bass2jax.run_bass_via_pjrt)
# ----------------------------------------------------------------------------

def _make_runner(ni_core=NI_CORE, n_cores=NCORES):
    import jax
    import numpy as _np
    from jax.sharding import Mesh, PartitionSpec
    try:
        from jax.experimental.shard_map import shard_map
    except Exception:
        from jax.shard_map import shard_map  # newer jax
    import concourse.mybir as mybir
    from concourse import bass2jax

    nc = _build_nc(ni_core)
    bass2jax.install_neuronx_cc_hook()

    pid_name = nc.partition_id_tensor.name if nc.partition_id_tensor else None
    in_names, out_names, out_avals = [], [], []
    for alloc in nc.m.functions[0].allocations:
        if not isinstance(alloc, mybir.MemoryLocationSet):
            continue
        name = alloc.memorylocations[0].name
        if alloc.kind == "ExternalInput":
            if name != pid_name:
                in_names.append(name)
        elif alloc.kind == "ExternalOutput":
            out_names.append(name)
            out_avals.append(jax.core.ShapedArray(
                tuple(alloc.tensor_shape), mybir.dt.np(alloc.dtype)))
    n_params = len(in_names)
    all_names = in_names + out_names
    if pid_name is not None:
        all_names = all_names + [pid_name]

    def _body(*args):
        operands = list(args)
        if pid_name is not None:
            operands.append(bass2jax.partition_id_tensor())
        outs = bass2jax._bass_exec_p.bind(
            *operands,
            out_avals=tuple(out_avals),
            in_names=tuple(all_names),
            out_names=tuple(out_names),
            lowering_input_output_aliases=(),
            sim_require_finite=True,
            sim_require_nnan=True,
            nc=nc,
        )
        return tuple(outs)

    devices = jax.devices()[:n_cores]
    mesh = Mesh(_np.asarray(devices), ("core",))
    donate = tuple(range(n_params, n_params + len(out_names)))
    sharded = jax.jit(
        shard_map(_body, mesh=mesh,
                  in_specs=(PartitionSpec("core"),) * (n_params + len(out_names)),
                  out_specs=(PartitionSpec("core"),) * len(out_names),
                  check_rep=False),
        donate_argnums=donate, keep_unused=True)

    def run(in_maps):
        per_core = [[_np.asarray(m[nm]) for nm in in_names] for m in in_maps]
        concat_in = [_np.concatenate([per_core[c][i] for c in range(n_cores)], 0)
                     for i in range(n_params)]
        concat_zeros = [
            _np.zeros((n_cores * a.shape[0], *a.shape[1:]), a.dtype)
            for a in out_avals]
        out_arrs = sharded(*concat_in, *concat_zeros)
        return [
            {nm: _np.asarray(out_arrs[i]).reshape(n_cores, *out_avals[i].shape)[c]
             for i, nm in enumerate(out_names)}
            for c in range(n_cores)
        ], out_arrs

    return run, in_names


def make_in_maps(inputs, ni_core=NI_CORE, n_cores=NCORES):
    f = lambda k: np.ascontiguousarray(np.asarray(inputs[k], dtype=np.float32))
    pos, vel, mass = f("positions"), f("velocities"), f("mass")
    ext, ela, fri = f("external_forces"), f("elasticity"), f("friction")
    base = {
        "posT": np.ascontiguousarray(pos.T),
        "velT": np.ascontiguousarray(vel.T),
        "massR": mass.reshape(1, N),
        "W1": f("W1"), "b1": f("b1").reshape(1, 64),
        "W2": f("W2"), "b2": f("b2").reshape(1, 64),
        "W3": f("W3"), "b3": f("b3").reshape(1, 32),
        "W4": f("W4"), "b4": f("b4").reshape(1, 3),
    }
    maps = []
    for c in range(n_cores):
        s = slice(c * ni_core, (c + 1) * ni_core)
        maps.append(dict(
            base,
            pos_my=np.ascontiguousarray(pos[s]),
            vel_my=np.ascontiguousarray(vel[s]),
            mass_my=np.ascontiguousarray(mass[s]).reshape(ni_core, 1),
            ext_my=np.ascontiguousarray(ext[s]),
            ela_my=np.ascontiguousarray(ela[s]).reshape(ni_core, 1),
            fri_my=np.ascontiguousarray(fri[s]).reshape(ni_core, 1),
        ))
    return maps


def get_runner():
    global _RUNNER
    if _RUNNER is None:
        _RUNNER = _make_runner()
    return _RUNNER


def kernel(**inputs):
    run, _ = get_runner()
    results, _ = run(make_in_maps(inputs))
    new_pos = np.concatenate([results[c]["out_pos"] for c in range(NCORES)], 0)
    new_vel = np.concatenate([results[c]["out_vel"] for c in range(NCORES)], 0)
    return new_pos, new_vel



# revision 8
# speedup vs baseline: 1.5035x; 1.5035x over previous
"""Trainium2 Bass kernel for nn_DifferentiableParticleSystem (N=2048 GNN message passing).

Strategy (8 NeuronCores, shard receiver rows i):
  - Each core owns 256 receiver particles i; positions/velocities/mass replicated.
  - Edge features are generated in [i-partition, j-free] plane layout (full 128-lane
    DVE utilization), gathered per-i into [8, 2048] feature-major layout via DMA.
  - The 4-layer edge MLP runs on the PE as 2 block-diagonal fp16 matmuls per chunk:
      pass X = L1 (feat->h1) + L3 (h2->h3),  pass Y = L2 (h1->h2) + L4 (h3->f)
  - The 2048 j-columns are split into 4 independent 512-col chains, one PSUM bank
    each for X and Y. Chains only couple through the per-row gather/scatter DMAs,
    so the Act relu / DVE bias+relu ops of different chains overlap instead of
    serializing on the cross-row dependency (which bounded the old pipeline).
  - relu(X) on ScalarE, relu(Y)+bias on VectorE; raw z4 (pre-tanh) is scattered
    back to [i, j] planes; the (dist<1) mask multiplies z4 (tanh(0)=0 keeps masked
    edges dead), tanh+accumulate on ScalarE reduces over j in one pass. The
    diagonal (j==i) term is removed by subtracting 10*tanh(MLP([0..0,1])).
  - Particle integration epilogue runs per 128-i half in [i-partition, 3] layout.
"""

import numpy as np

N = 2048
NCORES = 8
NI_CORE = N // NCORES  # 256 receiver rows per core
DT_STEP = 0.016
GRAV_Y = -9.8

_RUNNER = None


# ----------------------------------------------------------------------------
# Bass program
# ----------------------------------------------------------------------------

def _build_nc(ni_core=NI_CORE):
    import concourse.bacc as bacc
    import concourse.mybir as mybir
    from concourse.tile import TileContext
    from concourse.mybir import AluOpType as op

    f32 = mybir.dt.float32
    f16 = mybir.dt.float16
    AF = mybir.ActivationFunctionType
    AX = mybir.AxisListType

    J = N              # sender dimension (full)
    S = 4              # independent j-chains
    FD = J // S        # 512 free columns per chain
    R = 6              # rx ring depth
    G = 3              # gather prefetch distance
    halves = [(h * 128, min(128, ni_core - h * 128))
              for h in range((ni_core + 127) // 128)]

    nc = bacc.Bacc("TRN2", target_bir_lowering=False)

    # ---- DRAM I/O ----
    posT = nc.dram_tensor("posT", [3, J], f32, kind="ExternalInput")
    velT = nc.dram_tensor("velT", [3, J], f32, kind="ExternalInput")
    massR = nc.dram_tensor("massR", [1, J], f32, kind="ExternalInput")
    pos_my = nc.dram_tensor("pos_my", [ni_core, 3], f32, kind="ExternalInput")
    vel_my = nc.dram_tensor("vel_my", [ni_core, 3], f32, kind="ExternalInput")
    mass_my = nc.dram_tensor("mass_my", [ni_core, 1], f32, kind="ExternalInput")
    ext_my = nc.dram_tensor("ext_my", [ni_core, 3], f32, kind="ExternalInput")
    ela_my = nc.dram_tensor("ela_my", [ni_core, 1], f32, kind="ExternalInput")
    fri_my = nc.dram_tensor("fri_my", [ni_core, 1], f32, kind="ExternalInput")
    W1d = nc.dram_tensor("W1", [8, 64], f32, kind="ExternalInput")
    b1d = nc.dram_tensor("b1", [1, 64], f32, kind="ExternalInput")
    W2d = nc.dram_tensor("W2", [64, 64], f32, kind="ExternalInput")
    b2d = nc.dram_tensor("b2", [1, 64], f32, kind="ExternalInput")
    W3d = nc.dram_tensor("W3", [64, 32], f32, kind="ExternalInput")
    b3d = nc.dram_tensor("b3", [1, 32], f32, kind="ExternalInput")
    W4d = nc.dram_tensor("W4", [32, 3], f32, kind="ExternalInput")
    b4d = nc.dram_tensor("b4", [1, 3], f32, kind="ExternalInput")
    out_pos = nc.dram_tensor("out_pos", [ni_core, 3], f32, kind="ExternalOutput")
    out_vel = nc.dram_tensor("out_vel", [ni_core, 3], f32, kind="ExternalOutput")

    # ---- persistent SBUF ----
    def sb(name, shape, dt):
        return nc.alloc_sbuf_tensor(name, shape, dt)

    pj = [sb(f"pj{d}", [128, J], f16) for d in range(3)]     # pos_j bcast planes
    vj = [sb(f"vj{d}", [128, J], f16) for d in range(3)]     # vel_j bcast planes
    imj = sb("imj", [128, J], f16)                           # 1/mass_j bcast
    FP = [sb(f"fp{h}", [128, 8 * J], f16) for h in range(len(halves))]
    ZP = [sb(f"zp{h}", [128, 3 * J], f16) for h in range(len(halves))]
    scrA = sb("scrA", [128, J], f16)
    scrB = sb("scrB", [128, J], f16)
    maskS = sb("maskS", [128, J], f16)
    lhsTX = sb("lhsTX", [104, 96], f16)      # W3_hi @ rows 0-63, W1_hi @ 96-103
    lhsTXl = sb("lhsTXl", [104, 96], f16)    # W3_lo / W1_lo residuals
    lhsTY = sb("lhsTY", [96, 67], f16)       # W2_hi @ rows 0-63, W4_hi @ 64-95
    lhsTYl = sb("lhsTYl", [96, 67], f16)     # W2_lo / W4_lo residuals
    RX = [sb(f"rx{r}", [104, J], f16) for r in range(R)]     # rhs ring for pass X
    relucol = sb("relucol", [67, 1], f32)                    # 0 / -inf max operand
    b13col = sb("b13col", [96, 1], f32)                      # [b1; b3] relu-X bias
    byc = sb("byc", [67, 1], f32)                            # [b2; b4] pass-Y bias
    RY = [sb(f"ry{r}", [96, J], f16) for r in range(2)]      # rhs ring for pass Y
    negfd = sb("negfd", [128, 3], f32)                       # -10*tanh(z4_diag)
    nf = [sb(f"nf{h}", [128, 3], f32) for h in range(len(halves))]

    Xp = [nc.alloc_psum_tensor(f"xps{s}", [96, FD], f32) for s in range(S)]
    Yp = [nc.alloc_psum_tensor(f"yps{s}", [67, FD], f32) for s in range(S)]

    with TileContext(nc) as tc:
        import contextlib
        with contextlib.ExitStack() as ctx:
            pool = ctx.enter_context(tc.tile_pool(name="misc", bufs=1))

            # ================= setup: weights (fp16) =================
            nc.vector.memset(lhsTX.ap()[:, :], 0.0)
            nc.vector.memset(lhsTXl.ap()[:, :], 0.0)
            nc.vector.memset(lhsTY.ap()[:, :], 0.0)
            nc.vector.memset(lhsTYl.ap()[:, :], 0.0)

            def load_w(dram_ap, rows, cols, hi_t, lo_t):
                nr, ncol = rows.stop - rows.start, cols.stop - cols.start
                w32 = pool.tile([64, 64], f32, tag="w32s")
                nc.sync.dma_start(w32[0:nr, 0:ncol], dram_ap)
                hi16 = pool.tile([64, 64], f16, tag="w16h")
                nc.vector.tensor_copy(hi16[0:nr, 0:ncol], w32[0:nr, 0:ncol])
                nc.vector.tensor_copy(hi_t.ap()[rows, cols], hi16[0:nr, 0:ncol])
                res = pool.tile([64, 64], f32, tag="w32r")
                nc.vector.tensor_tensor(res[0:nr, 0:ncol], w32[0:nr, 0:ncol],
                                        hi16[0:nr, 0:ncol], op.subtract)
                nc.vector.tensor_copy(lo_t.ap()[rows, cols], res[0:nr, 0:ncol])

            load_w(W1d[:, :], slice(96, 104), slice(0, 64), lhsTX, lhsTXl)
            load_w(W3d[:, :], slice(0, 64), slice(64, 96), lhsTX, lhsTXl)
            load_w(W2d[:, :], slice(0, 64), slice(0, 64), lhsTY, lhsTYl)
            load_w(W4d[:, :], slice(64, 96), slice(64, 67), lhsTY, lhsTYl)
            # bias columns ([1,n] dram rows -> [n,1] sbuf cols via swdge dma)
            nc.gpsimd.dma_start(b13col.ap()[0:64, 0:1], b1d[0:1, 0:64])
            nc.gpsimd.dma_start(b13col.ap()[64:96, 0:1], b3d[0:1, 0:32])
            nc.gpsimd.dma_start(byc.ap()[0:64, 0:1], b2d[0:1, 0:64])
            nc.gpsimd.dma_start(byc.ap()[64:67, 0:1], b4d[0:1, 0:3])

            # ================= setup: broadcast planes =================
            for d in range(3):
                r32 = pool.tile([1, J], f32, tag="rowstg32")
                r16 = pool.tile([1, J], f16, tag="rowstg16")
                nc.sync.dma_start(r32[:, :], posT[d:d + 1, :])
                nc.vector.tensor_copy(r16[:, :], r32[:, :])
                nc.gpsimd.partition_broadcast(pj[d].ap()[:, :], r16[0:1, :])
                r32v = pool.tile([1, J], f32, tag="rowstg32")
                r16v = pool.tile([1, J], f16, tag="rowstg16")
                nc.sync.dma_start(r32v[:, :], velT[d:d + 1, :])
                nc.vector.tensor_copy(r16v[:, :], r32v[:, :])
                nc.gpsimd.partition_broadcast(vj[d].ap()[:, :], r16v[0:1, :])
            stgm = pool.tile([1, J], f32, tag="rowstg32")
            stgm16 = pool.tile([1, J], f16, tag="rowstg16")
            nc.sync.dma_start(stgm[:, :], massR[:, :])
            nc.vector.reciprocal(stgm[:, :], stgm[:, :])
            nc.vector.tensor_copy(stgm16[:, :], stgm[:, :])
            nc.gpsimd.partition_broadcast(imj.ap()[:, :], stgm16[0:1, :])

            # ================= setup: ring constants =================
            for r in range(R):
                nc.gpsimd.memset(RX[r].ap()[64:96, :], 0.0)
            nc.gpsimd.memset(RX[0].ap()[0:64, :], 0.0)
            nc.gpsimd.memset(relucol.ap()[0:64, :], 0.0)
            nc.gpsimd.memset(relucol.ap()[64:67, :], -1e30)

            # ================= feature planes =================
            def feat_ops(h):
                """Thunk list writing FP[h]; executed lazily to interleave."""
                i0, Ph = halves[h]
                ops = []

                def posvel(d):
                    pc = pool.tile([128, 1], f32, tag=f"pic{h}{d}")
                    nc.sync.dma_start(pc[0:Ph, :], pos_my[i0:i0 + Ph, d:d + 1])
                    nc.vector.tensor_scalar(
                        FP[h].ap()[0:Ph, d * J:(d + 1) * J], pj[d].ap()[0:Ph, :],
                        pc[0:Ph, :], None, op.subtract)
                    vc = pool.tile([128, 1], f32, tag=f"vic{h}{d}")
                    nc.sync.dma_start(vc[0:Ph, :], vel_my[i0:i0 + Ph, d:d + 1])
                    nc.vector.tensor_scalar(
                        FP[h].ap()[0:Ph, (3 + d) * J:(4 + d) * J], vj[d].ap()[0:Ph, :],
                        vc[0:Ph, :], None, op.subtract)
                for d in range(3):
                    ops.append(lambda d=d: posvel(d))

                rel = lambda d: FP[h].ap()[0:Ph, d * J:(d + 1) * J]

                def dist2a():
                    nc.vector.tensor_tensor(scrA.ap()[0:Ph, :], rel(0), rel(0), op.mult)
                    nc.vector.tensor_tensor(scrB.ap()[0:Ph, :], rel(1), rel(1), op.mult)
                    nc.vector.tensor_tensor(scrA.ap()[0:Ph, :], scrA.ap()[0:Ph, :],
                                            scrB.ap()[0:Ph, :], op.add)

                def dist2b():
                    nc.vector.tensor_tensor(scrB.ap()[0:Ph, :], rel(2), rel(2), op.mult)
                    nc.vector.tensor_tensor(scrA.ap()[0:Ph, :], scrA.ap()[0:Ph, :],
                                            scrB.ap()[0:Ph, :], op.add)
                    nc.scalar.activation(FP[h].ap()[0:Ph, 6 * J:7 * J],
                                         scrA.ap()[0:Ph, :], AF.Sqrt)

                def ratio():
                    mc = pool.tile([128, 1], f32, tag=f"mic{h}")
                    nc.sync.dma_start(mc[0:Ph, :], mass_my[i0:i0 + Ph, :])
                    nc.vector.tensor_scalar(FP[h].ap()[0:Ph, 7 * J:8 * J],
                                            imj.ap()[0:Ph, :], mc[0:Ph, :], None,
                                            op.mult)
                ops.extend([dist2a, dist2b, ratio])
                return ops

            for f in feat_ops(0):
                f()
            pending = feat_ops(1) if len(halves) > 1 else []

            # ================= f_diag (on-device, same weights) =================
            fdx1 = pool.tile([104, 1], f16, tag="fdx1")
            nc.gpsimd.memset(fdx1[0:96, :], 0.0)
            r9a = pool.tile([1, 8], f16, tag="r9a")
            nc.gpsimd.memset(r9a[:, :], 0.0)
            nc.gpsimd.memset(r9a[0:1, 7:8], 1.0)  # ratio feature = 1
            nc.gpsimd.dma_start(fdx1[96:104, 0:1], r9a[0:1, 0:8])
            def mm_x1(out_ap, rhs_ap):
                nc.tensor.matmul(out_ap, lhsTX.ap()[:, :], rhs_ap, start=True,
                                 stop=False)
                nc.tensor.matmul(out_ap, lhsTXl.ap()[:, :], rhs_ap, start=False,
                                 stop=True)

            def mm_y1(out_ap, rhs_ap):
                nc.tensor.matmul(out_ap, lhsTY.ap()[:, :], rhs_ap, start=True,
                                 stop=False)
                nc.tensor.matmul(out_ap, lhsTYl.ap()[:, :], rhs_ap, start=False,
                                 stop=True)

            mm_x1(Xp[0].ap()[0:96, 0:1], fdx1[0:104, :])
            fdy1 = pool.tile([96, 1], f16, tag="fdy1")
            nc.scalar.activation(fdy1[0:96, :], Xp[0].ap()[0:96, 0:1], AF.Relu,
                                 bias=b13col.ap()[0:96, :])
            mm_y1(Yp[0].ap()[0:67, 0:1], fdy1[0:96, :])
            fdx2 = pool.tile([104, 1], f16, tag="fdx2")
            nc.gpsimd.memset(fdx2[64:104, :], 0.0)
            nc.vector.tensor_scalar(fdx2[0:64, :], Yp[0].ap()[0:64, 0:1],
                                    byc.ap()[0:64, :], relucol.ap()[0:64, :],
                                    op.add, op.max)
            mm_x1(Xp[0].ap()[0:96, 0:1], fdx2[0:104, :])
            fdy2 = pool.tile([96, 1], f16, tag="fdy2")
            nc.scalar.activation(fdy2[0:96, :], Xp[0].ap()[0:96, 0:1], AF.Relu,
                                 bias=b13col.ap()[0:96, :])
            mm_y1(Yp[0].ap()[0:67, 0:1], fdy2[0:96, :])
            fd3 = pool.tile([3, 1], f32, tag="fd3")
            nc.scalar.activation(fd3[:, :], Yp[0].ap()[64:67, 0:1], AF.Tanh,
                                 bias=byc.ap()[64:67, :])
            nc.vector.tensor_scalar(fd3[:, :], fd3[:, :], -10.0, None, op.mult)
            fdrow = pool.tile([1, 3], f32, tag="fdrow")
            nc.gpsimd.dma_start(fdrow[0:1, 0:3], fd3[0:3, 0:1])
            nc.gpsimd.partition_broadcast(negfd.ap()[:, :], fdrow[0:1, :])

            # ================= post pass (mask, tanh-reduce, integrate) =========
            def do_post(h):
                i0, Ph = halves[h]
                nc.vector.tensor_scalar(maskS.ap()[0:Ph, :],
                                        FP[h].ap()[0:Ph, 6 * J:7 * J],
                                        1.0, None, op.is_lt)
                for d in range(3):
                    nc.vector.tensor_tensor(scrA.ap()[0:Ph, :],
                                            ZP[h].ap()[0:Ph, d * J:(d + 1) * J],
                                            maskS.ap()[0:Ph, :], op.mult)
                    nc.scalar.activation(scrB.ap()[0:Ph, :], scrA.ap()[0:Ph, :],
                                         AF.Tanh,
                                         accum_out=nf[h].ap()[0:Ph, d:d + 1])
                # nf_total = 10*sum + negfd  (negfd = -10*tanh(z4_diag))
                nc.vector.scalar_tensor_tensor(nf[h].ap()[0:Ph, :],
                                               nf[h].ap()[0:Ph, :], 10.0,
                                               negfd.ap()[0:Ph, :],
                                               op.mult, op.add)
                # ---- particle update epilogue ----
                ext = pool.tile([128, 3], f32, tag=f"ext{h}")
                nc.sync.dma_start(ext[0:Ph, :], ext_my[i0:i0 + Ph, :])
                pcol = pool.tile([128, 3], f32, tag=f"pc3{h}")
                nc.sync.dma_start(pcol[0:Ph, :], pos_my[i0:i0 + Ph, :])
                vcol = pool.tile([128, 3], f32, tag=f"vc3{h}")
                nc.sync.dma_start(vcol[0:Ph, :], vel_my[i0:i0 + Ph, :])
                mcol = pool.tile([128, 1], f32, tag=f"mm{h}")
                nc.sync.dma_start(mcol[0:Ph, :], mass_my[i0:i0 + Ph, :])
                ecol = pool.tile([128, 1], f32, tag=f"ee{h}")
                nc.sync.dma_start(ecol[0:Ph, :], ela_my[i0:i0 + Ph, :])
                fcol = pool.tile([128, 1], f32, tag=f"ff{h}")
                nc.sync.dma_start(fcol[0:Ph, :], fri_my[i0:i0 + Ph, :])

                frc = pool.tile([128, 3], f32, tag=f"frc{h}")
                nc.vector.tensor_tensor(frc[0:Ph, :], nf[h].ap()[0:Ph, :],
                                        ext[0:Ph, :], op.add)
                nc.vector.scalar_tensor_tensor(frc[0:Ph, 1:2], mcol[0:Ph, :],
                                               GRAV_Y, frc[0:Ph, 1:2],
                                               op.mult, op.add)
                imc = pool.tile([128, 1], f32, tag=f"imc{h}")
                nc.vector.reciprocal(imc[0:Ph, :], mcol[0:Ph, :])
                nc.vector.tensor_scalar(imc[0:Ph, :], imc[0:Ph, :], DT_STEP, None,
                                        op.mult)
                v1 = pool.tile([128, 3], f32, tag=f"v1{h}")
                nc.vector.scalar_tensor_tensor(v1[0:Ph, :], frc[0:Ph, :],
                                               imc[0:Ph, :], vcol[0:Ph, :],
                                               op.mult, op.add)
                sq = pool.tile([128, 3], f32, tag=f"sq{h}")
                nc.vector.tensor_tensor(sq[0:Ph, :], v1[0:Ph, :], v1[0:Ph, :],
                                        op.mult)
                spd = pool.tile([128, 1], f32, tag=f"spd{h}")
                nc.vector.reduce_sum(spd[0:Ph, :], sq[0:Ph, :], AX.X)
                nc.scalar.activation(spd[0:Ph, :], spd[0:Ph, :], AF.Sqrt)
                fm = pool.tile([128, 1], f32, tag=f"fm{h}")
                nc.vector.tensor_scalar(fm[0:Ph, :], spd[0:Ph, :], 0.1, None,
                                        op.is_gt)
                ffac = pool.tile([128, 1], f32, tag=f"ffac{h}")
                nc.vector.tensor_tensor(ffac[0:Ph, :], fcol[0:Ph, :], fm[0:Ph, :],
                                        op.mult)
                nc.vector.tensor_scalar(ffac[0:Ph, :], ffac[0:Ph, :], -DT_STEP, 1.0,
                                        op.mult, op.add)
                nc.vector.tensor_scalar(v1[0:Ph, :], v1[0:Ph, :], ffac[0:Ph, :],
                                        None, op.mult)
                p2 = pool.tile([128, 3], f32, tag=f"p2{h}")
                nc.vector.scalar_tensor_tensor(p2[0:Ph, :], v1[0:Ph, :], DT_STEP,
                                               pcol[0:Ph, :], op.mult, op.add)
                cm = pool.tile([128, 1], f32, tag=f"cm{h}")
                nc.vector.tensor_scalar(cm[0:Ph, :], p2[0:Ph, 1:2], 0.0, None,
                                        op.is_lt)
                ncm = pool.tile([128, 1], f32, tag=f"ncm{h}")
                nc.vector.tensor_scalar(ncm[0:Ph, :], cm[0:Ph, :], -1.0, 1.0,
                                        op.mult, op.add)
                nc.vector.tensor_tensor(p2[0:Ph, 1:2], p2[0:Ph, 1:2], ncm[0:Ph, :],
                                        op.mult)
                w1e = pool.tile([128, 1], f32, tag=f"w1e{h}")
                nc.vector.tensor_scalar(w1e[0:Ph, :], ecol[0:Ph, :], 1.0, None,
                                        op.add)
                nc.vector.tensor_tensor(w1e[0:Ph, :], cm[0:Ph, :], w1e[0:Ph, :],
                                        op.mult)
                nc.vector.tensor_scalar(w1e[0:Ph, :], w1e[0:Ph, :], -1.0, 1.0,
                                        op.mult, op.add)
                nc.vector.tensor_tensor(v1[0:Ph, 1:2], v1[0:Ph, 1:2], w1e[0:Ph, :],
                                        op.mult)
                nc.sync.dma_start(out_pos[i0:i0 + Ph, :], p2[0:Ph, :])
                nc.sync.dma_start(out_vel[i0:i0 + Ph, :], v1[0:Ph, :])

            # ================= main pipeline =================
            def gather(kk):
                h, p = (0, kk) if kk < 128 else (1, kk - 128)
                src = FP[h].ap()[p:p + 1, :].rearrange("p (d j) -> p d j", d=8)
                nc.gpsimd.dma_start(RX[kk % R].ap()[96:104, :], src)

            for kk in range(min(G, ni_core)):
                gather(kk)

            # lag-1 modulo schedule, chain-major: in round k, each 1024-col
            # pair P finishes row k-1 (relu / Y-matmuls / bias+relu) and then
            # starts row k (X-matmuls). This keeps next-row X work adjacent to
            # current-row Y work in the PE queue, so no engine FIFO-blocks.
            for k in range(ni_core + 2):
                rx = RX[k % R].ap()
                ry = RY[(k - 1) % 2].ap()
                rxn = RX[k % R].ap()  # tsp(k-1) writes slot ((k-1)+1)%R == k%R
                if k + G < ni_core:
                    gather(k + G)
                if 1 <= k <= ni_core + 1:
                    for s in range(S):
                        sl = slice(s * FD, (s + 1) * FD)
                        nc.scalar.activation(ry[0:96, sl], Xp[s].ap()[0:96, :],
                                             AF.Relu, bias=b13col.ap()[0:96, :])
                    for s in range(S):
                        sl = slice(s * FD, (s + 1) * FD)
                        for lhs, st in ((lhsTY, True), (lhsTYl, False)):
                            nc.tensor.matmul(Yp[s].ap()[0:67, :], lhs.ap()[:, :],
                                             ry[0:96, sl], start=st, stop=not st)
                    for s in range(S):
                        sl = slice(s * FD, (s + 1) * FD)
                        nc.vector.tensor_scalar(rxn[0:67, sl], Yp[s].ap()[0:67, :],
                                                byc.ap()[0:67, :],
                                                relucol.ap()[0:67, :],
                                                op.add, op.max)
                if k <= ni_core:
                    for s in range(S):
                        sl = slice(s * FD, (s + 1) * FD)
                        for lhs, st in ((lhsTX, True), (lhsTXl, False)):
                            nc.tensor.matmul(Xp[s].ap()[0:96, :], lhs.ap()[:, :],
                                             rx[0:104, sl], start=st, stop=not st)
                if k >= 2:
                    i = k - 2
                    h, p = (0, i) if i < 128 else (1, i - 128)
                    dst = ZP[h].ap()[p:p + 1, :].rearrange("p (d j) -> p d j", d=3)
                    src_slot = RX[(i + 2) % R].ap()
                    nc.sync.dma_start(dst, src_slot[64:67, :])
                # interleave half-1 feature generation into early rounds
                if pending and k % 8 == 5:
                    pending.pop(0)()
                if k == 133 and len(halves) > 1:
                    while pending:
                        pending.pop(0)()
                    do_post(0)
            while pending:
                pending.pop(0)()
            do_post(len(halves) - 1)

    nc.compile()
    return nc


# ----------------------------------------------------------------------------
# Host-side runner (persistent jitted executable, modeled on
# concourse.bass2jax.run_bass_via_pjrt)
# ----------------------------------------------------------------------------

def _make_runner(ni_core=NI_CORE, n_cores=NCORES):
    import jax
    import numpy as _np
    from jax.sharding import Mesh, PartitionSpec
    try:
        from jax.experimental.shard_map import shard_map
    except Exception:
        from jax.shard_map import shard_map  # newer jax
    import concourse.mybir as mybir
    from concourse import bass2jax

    nc = _build_nc(ni_core)
    bass2jax.install_neuronx_cc_hook()

    pid_name = nc.partition_id_tensor.name if nc.partition_id_tensor else None
    in_names, out_names, out_avals = [], [], []
    for alloc in nc.m.functions[0].allocations:
        if not isinstance(alloc, mybir.MemoryLocationSet):
            continue
        name = alloc.memorylocations[0].name
        if alloc.kind == "ExternalInput":
            if name != pid_name:
                in_names.append(name)
        elif alloc.kind == "ExternalOutput":
            out_names.append(name)
            out_avals.append(jax.core.ShapedArray(
                tuple(alloc.tensor_shape), mybir.dt.np(alloc.dtype)))
    n_params = len(in_names)
    all_names = in_names + out_names
    if pid_name is not None:
        all_names = all_names + [pid_name]

    def _body(*args):
        operands = list(args)
        if pid_name is not None:
            operands.append(bass2jax.partition_id_tensor())
        outs = bass2jax._bass_exec_p.bind(
            *operands,
            out_avals=tuple(out_avals),
            in_names=tuple(all_names),
            out_names=tuple(out_names),
            lowering_input_output_aliases=(),
            sim_require_finite=True,
            sim_require_nnan=True,
            nc=nc,
        )
        return tuple(outs)

    devices = jax.devices()[:n_cores]
    mesh = Mesh(_np.asarray(devices), ("core",))
    donate = tuple(range(n_params, n_params + len(out_names)))
    sharded = jax.jit(
        shard_map(_body, mesh=mesh,
                  in_specs=(PartitionSpec("core"),) * (n_params + len(out_names)),
                  out_specs=(PartitionSpec("core"),) * len(out_names),
                  check_rep=False),
        donate_argnums=donate, keep_unused=True)

    def run(in_maps):
        per_core = [[_np.asarray(m[nm]) for nm in in_names] for m in in_maps]
        concat_in = [_np.concatenate([per_core[c][i] for c in range(n_cores)], 0)
                     for i in range(n_params)]
        concat_zeros = [
            _np.zeros((n_cores * a.shape[0], *a.shape[1:]), a.dtype)
            for a in out_avals]
        out_arrs = sharded(*concat_in, *concat_zeros)
        return [
            {nm: _np.asarray(out_arrs[i]).reshape(n_cores, *out_avals[i].shape)[c]
             for i, nm in enumerate(out_names)}
            for c in range(n_cores)
        ], out_arrs

    return run, in_names


def make_in_maps(inputs, ni_core=NI_CORE, n_cores=NCORES):
    f = lambda k: np.ascontiguousarray(np.asarray(inputs[k], dtype=np.float32))
    pos, vel, mass = f("positions"), f("velocities"), f("mass")
    ext, ela, fri = f("external_forces"), f("elasticity"), f("friction")
    base = {
        "posT": np.ascontiguousarray(pos.T),
        "velT": np.ascontiguousarray(vel.T),
        "massR": mass.reshape(1, N),
        "W1": f("W1"), "b1": f("b1").reshape(1, 64),
        "W2": f("W2"), "b2": f("b2").reshape(1, 64),
        "W3": f("W3"), "b3": f("b3").reshape(1, 32),
        "W4": f("W4"), "b4": f("b4").reshape(1, 3),
    }
    maps = []
    for c in range(n_cores):
        s = slice(c * ni_core, (c + 1) * ni_core)
        maps.append(dict(
            base,
            pos_my=np.ascontiguousarray(pos[s]),
            vel_my=np.ascontiguousarray(vel[s]),
            mass_my=np.ascontiguousarray(mass[s]).reshape(ni_core, 1),
            ext_my=np.ascontiguousarray(ext[s]),
            ela_my=np.ascontiguousarray(ela[s]).reshape(ni_core, 1),
            fri_my=np.ascontiguousarray(fri[s]).reshape(ni_core, 1),
        ))
    return maps


def get_runner():
    global _RUNNER
    if _RUNNER is None:
        _RUNNER = _make_runner()
    return _RUNNER


def kernel(**inputs):
    run, _ = get_runner()
    results, _ = run(make_in_maps(inputs))
    new_pos = np.concatenate([results[c]["out_pos"] for c in range(NCORES)], 0)
    new_vel = np.concatenate([results[c]["out_vel"] for c in range(NCORES)], 0)
    return new_pos, new_vel


# revision 10
# speedup vs baseline: 2.0013x; 1.3311x over previous
"""Trainium2 Bass kernel for nn_DifferentiableParticleSystem (N=2048 GNN message passing).

Strategy (8 NeuronCores, shard receiver rows i):
  - Each core owns 256 receiver particles i; positions/velocities/mass replicated.
  - Edge features are generated in [i-partition, j-free] plane layout (full 128-lane
    DVE utilization), gathered per-i into [8, 2048] feature-major layout via DMA.
  - The 4-layer edge MLP runs on the PE as 2 block-diagonal fp16 matmuls per chunk:
      pass X = L1 (feat->h1) + L3 (h2->h3),  pass Y = L2 (h1->h2) + L4 (h3->f)
  - The 2048 j-columns are split into 4 independent 512-col chains, one PSUM bank
    each for X and Y. Chains only couple through the per-row gather/scatter DMAs,
    so the Act relu / DVE bias+relu ops of different chains overlap instead of
    serializing on the cross-row dependency (which bounded the old pipeline).
  - relu(X) on ScalarE, relu(Y)+bias on VectorE; raw z4 (pre-tanh) is scattered
    back to [i, j] planes; the (dist<1) mask multiplies z4 (tanh(0)=0 keeps masked
    edges dead), tanh+accumulate on ScalarE reduces over j in one pass. The
    diagonal (j==i) term is removed by subtracting 10*tanh(MLP([0..0,1])).
  - Particle integration epilogue runs per 128-i half in [i-partition, 3] layout.
"""

import numpy as np

N = 2048
NCORES = 8
NI_CORE = N // NCORES  # 256 receiver rows per core
DT_STEP = 0.016
GRAV_Y = -9.8

_RUNNER = None


# ----------------------------------------------------------------------------
# Bass program
# ----------------------------------------------------------------------------

def _build_nc(ni_core=NI_CORE):
    import concourse.bacc as bacc
    import concourse.mybir as mybir
    from concourse.tile import TileContext
    from concourse.mybir import AluOpType as op

    f32 = mybir.dt.float32
    f16 = mybir.dt.float16
    AF = mybir.ActivationFunctionType
    AX = mybir.AxisListType

    J = N              # sender dimension (full)
    S = 4              # independent j-chains
    FD = J // S        # 512 free columns per chain
    R = 6              # rx ring depth
    G = 3              # gather prefetch distance
    halves = [(h * 128, min(128, ni_core - h * 128))
              for h in range((ni_core + 127) // 128)]

    nc = bacc.Bacc("TRN2", target_bir_lowering=False)

    # ---- DRAM I/O ----
    posT = nc.dram_tensor("posT", [3, J], f32, kind="ExternalInput")
    velT = nc.dram_tensor("velT", [3, J], f32, kind="ExternalInput")
    massR = nc.dram_tensor("massR", [1, J], f32, kind="ExternalInput")
    pos_my = nc.dram_tensor("pos_my", [ni_core, 3], f32, kind="ExternalInput")
    vel_my = nc.dram_tensor("vel_my", [ni_core, 3], f32, kind="ExternalInput")
    mass_my = nc.dram_tensor("mass_my", [ni_core, 1], f32, kind="ExternalInput")
    ext_my = nc.dram_tensor("ext_my", [ni_core, 3], f32, kind="ExternalInput")
    ela_my = nc.dram_tensor("ela_my", [ni_core, 1], f32, kind="ExternalInput")
    fri_my = nc.dram_tensor("fri_my", [ni_core, 1], f32, kind="ExternalInput")
    W1d = nc.dram_tensor("W1", [8, 64], f32, kind="ExternalInput")
    b1d = nc.dram_tensor("b1", [1, 64], f32, kind="ExternalInput")
    W2d = nc.dram_tensor("W2", [64, 64], f32, kind="ExternalInput")
    b2d = nc.dram_tensor("b2", [1, 64], f32, kind="ExternalInput")
    W3d = nc.dram_tensor("W3", [64, 32], f32, kind="ExternalInput")
    b3d = nc.dram_tensor("b3", [1, 32], f32, kind="ExternalInput")
    W4d = nc.dram_tensor("W4", [32, 3], f32, kind="ExternalInput")
    b4d = nc.dram_tensor("b4", [1, 3], f32, kind="ExternalInput")
    out_pos = nc.dram_tensor("out_pos", [ni_core, 3], f32, kind="ExternalOutput")
    out_vel = nc.dram_tensor("out_vel", [ni_core, 3], f32, kind="ExternalOutput")

    # ---- persistent SBUF ----
    def sb(name, shape, dt):
        return nc.alloc_sbuf_tensor(name, shape, dt)

    pj = [sb(f"pj{d}", [128, J], f16) for d in range(3)]     # pos_j bcast planes
    vj = [sb(f"vj{d}", [128, J], f16) for d in range(3)]     # vel_j bcast planes
    imj = sb("imj", [128, J], f16)                           # 1/mass_j bcast
    FP = [sb(f"fp{h}", [128, 8 * J], f16) for h in range(len(halves))]
    ZP = [sb(f"zp{h}", [128, 3 * J], f16) for h in range(len(halves))]
    scrA = sb("scrA", [128, J], f16)
    scrB = sb("scrB", [128, J], f16)
    maskS = sb("maskS", [128, J], f16)
    lhsTX = sb("lhsTX", [104, 96], f16)      # W3_hi @ rows 0-63, W1_hi @ 96-103
    lhsTXl = sb("lhsTXl", [104, 96], f16)    # W3_lo / W1_lo residuals
    lhsTY = sb("lhsTY", [96, 67], f16)       # W2_hi @ rows 0-63, W4_hi @ 64-95
    lhsTYl = sb("lhsTYl", [96, 67], f16)     # W2_lo / W4_lo residuals
    RX = [sb(f"rx{r}", [104, J], f16) for r in range(R)]     # rhs ring for pass X
    relucol = sb("relucol", [67, 1], f32)                    # 0 / -inf max operand
    b13col = sb("b13col", [96, 1], f32)                      # [b1; b3] relu-X bias
    byc = sb("byc", [67, 1], f32)                            # [b2; b4] pass-Y bias
    RY = [sb(f"ry{r}", [96, J], f16) for r in range(2)]      # rhs ring for pass Y
    negfd = sb("negfd", [128, 3], f32)                       # -10*tanh(z4_diag)
    nf = [sb(f"nf{h}", [128, 3], f32) for h in range(len(halves))]

    Xp = [nc.alloc_psum_tensor(f"xps{s}", [96, FD], f32) for s in range(S)]
    Yp = [nc.alloc_psum_tensor(f"yps{s}", [67, FD], f32) for s in range(S)]

    with TileContext(nc) as tc:
        import contextlib
        with contextlib.ExitStack() as ctx:
            pool = ctx.enter_context(tc.tile_pool(name="misc", bufs=1))

            # ================= setup: weights (fp16) =================
            nc.vector.memset(lhsTX.ap()[:, :], 0.0)
            nc.vector.memset(lhsTXl.ap()[:, :], 0.0)
            nc.vector.memset(lhsTY.ap()[:, :], 0.0)
            nc.vector.memset(lhsTYl.ap()[:, :], 0.0)

            def load_w(dram_ap, rows, cols, hi_t, lo_t):
                nr, ncol = rows.stop - rows.start, cols.stop - cols.start
                w32 = pool.tile([64, 64], f32, tag="w32s")
                nc.sync.dma_start(w32[0:nr, 0:ncol], dram_ap)
                hi16 = pool.tile([64, 64], f16, tag="w16h")
                nc.vector.tensor_copy(hi16[0:nr, 0:ncol], w32[0:nr, 0:ncol])
                nc.vector.tensor_copy(hi_t.ap()[rows, cols], hi16[0:nr, 0:ncol])
                res = pool.tile([64, 64], f32, tag="w32r")
                nc.vector.tensor_tensor(res[0:nr, 0:ncol], w32[0:nr, 0:ncol],
                                        hi16[0:nr, 0:ncol], op.subtract)
                nc.vector.tensor_copy(lo_t.ap()[rows, cols], res[0:nr, 0:ncol])

            load_w(W1d[:, :], slice(96, 104), slice(0, 64), lhsTX, lhsTXl)
            load_w(W3d[:, :], slice(0, 64), slice(64, 96), lhsTX, lhsTXl)
            load_w(W2d[:, :], slice(0, 64), slice(0, 64), lhsTY, lhsTYl)
            load_w(W4d[:, :], slice(64, 96), slice(64, 67), lhsTY, lhsTYl)
            # bias columns ([1,n] dram rows -> [n,1] sbuf cols via swdge dma)
            nc.gpsimd.dma_start(b13col.ap()[0:64, 0:1], b1d[0:1, 0:64])
            nc.gpsimd.dma_start(b13col.ap()[64:96, 0:1], b3d[0:1, 0:32])
            nc.gpsimd.dma_start(byc.ap()[0:64, 0:1], b2d[0:1, 0:64])
            nc.gpsimd.dma_start(byc.ap()[64:67, 0:1], b4d[0:1, 0:3])

            # ================= setup: broadcast planes =================
            for d in range(3):
                r32 = pool.tile([1, J], f32, tag="rowstg32")
                r16 = pool.tile([1, J], f16, tag="rowstg16")
                nc.sync.dma_start(r32[:, :], posT[d:d + 1, :])
                nc.vector.tensor_copy(r16[:, :], r32[:, :])
                nc.gpsimd.partition_broadcast(pj[d].ap()[:, :], r16[0:1, :])
                r32v = pool.tile([1, J], f32, tag="rowstg32")
                r16v = pool.tile([1, J], f16, tag="rowstg16")
                nc.sync.dma_start(r32v[:, :], velT[d:d + 1, :])
                nc.vector.tensor_copy(r16v[:, :], r32v[:, :])
                nc.gpsimd.partition_broadcast(vj[d].ap()[:, :], r16v[0:1, :])
            stgm = pool.tile([1, J], f32, tag="rowstg32")
            stgm16 = pool.tile([1, J], f16, tag="rowstg16")
            nc.sync.dma_start(stgm[:, :], massR[:, :])
            nc.vector.reciprocal(stgm[:, :], stgm[:, :])
            nc.vector.tensor_copy(stgm16[:, :], stgm[:, :])
            nc.gpsimd.partition_broadcast(imj.ap()[:, :], stgm16[0:1, :])

            # ================= setup: ring constants =================
            for r in range(R):
                nc.gpsimd.memset(RX[r].ap()[64:96, :], 0.0)
            nc.gpsimd.memset(RX[0].ap()[0:64, :], 0.0)
            nc.gpsimd.memset(relucol.ap()[0:64, :], 0.0)
            nc.gpsimd.memset(relucol.ap()[64:67, :], -1e30)

            # ================= feature planes =================
            def feat_ops(h):
                """Thunk list writing FP[h]; executed lazily to interleave."""
                i0, Ph = halves[h]
                ops = []

                def posvel(d):
                    pc = pool.tile([128, 1], f32, tag=f"pic{h}{d}")
                    nc.sync.dma_start(pc[0:Ph, :], pos_my[i0:i0 + Ph, d:d + 1])
                    nc.vector.tensor_scalar(
                        FP[h].ap()[0:Ph, d * J:(d + 1) * J], pj[d].ap()[0:Ph, :],
                        pc[0:Ph, :], None, op.subtract)
                    vc = pool.tile([128, 1], f32, tag=f"vic{h}{d}")
                    nc.sync.dma_start(vc[0:Ph, :], vel_my[i0:i0 + Ph, d:d + 1])
                    nc.vector.tensor_scalar(
                        FP[h].ap()[0:Ph, (3 + d) * J:(4 + d) * J], vj[d].ap()[0:Ph, :],
                        vc[0:Ph, :], None, op.subtract)
                for d in range(3):
                    ops.append(lambda d=d: posvel(d))

                rel = lambda d: FP[h].ap()[0:Ph, d * J:(d + 1) * J]

                def dist2a():
                    nc.vector.tensor_tensor(scrA.ap()[0:Ph, :], rel(0), rel(0), op.mult)
                    nc.vector.tensor_tensor(scrB.ap()[0:Ph, :], rel(1), rel(1), op.mult)
                    nc.vector.tensor_tensor(scrA.ap()[0:Ph, :], scrA.ap()[0:Ph, :],
                                            scrB.ap()[0:Ph, :], op.add)

                def dist2b():
                    nc.vector.tensor_tensor(scrB.ap()[0:Ph, :], rel(2), rel(2), op.mult)
                    nc.vector.tensor_tensor(scrA.ap()[0:Ph, :], scrA.ap()[0:Ph, :],
                                            scrB.ap()[0:Ph, :], op.add)
                    nc.scalar.activation(FP[h].ap()[0:Ph, 6 * J:7 * J],
                                         scrA.ap()[0:Ph, :], AF.Sqrt)

                def ratio():
                    mc = pool.tile([128, 1], f32, tag=f"mic{h}")
                    nc.sync.dma_start(mc[0:Ph, :], mass_my[i0:i0 + Ph, :])
                    nc.vector.tensor_scalar(FP[h].ap()[0:Ph, 7 * J:8 * J],
                                            imj.ap()[0:Ph, :], mc[0:Ph, :], None,
                                            op.mult)
                ops.extend([dist2a, dist2b, ratio])
                return ops

            for f in feat_ops(0):
                f()
            pending = feat_ops(1) if len(halves) > 1 else []

            # ================= f_diag (on-device, same weights) =================
            fdx1 = pool.tile([104, 1], f16, tag="fdx1")
            nc.gpsimd.memset(fdx1[0:96, :], 0.0)
            r9a = pool.tile([1, 8], f16, tag="r9a")
            nc.gpsimd.memset(r9a[:, :], 0.0)
            nc.gpsimd.memset(r9a[0:1, 7:8], 1.0)  # ratio feature = 1
            nc.gpsimd.dma_start(fdx1[96:104, 0:1], r9a[0:1, 0:8])
            def mm_x1(out_ap, rhs_ap):
                nc.tensor.matmul(out_ap, lhsTX.ap()[:, :], rhs_ap, start=True,
                                 stop=False)
                nc.tensor.matmul(out_ap, lhsTXl.ap()[:, :], rhs_ap, start=False,
                                 stop=True)

            def mm_y1(out_ap, rhs_ap):
                nc.tensor.matmul(out_ap, lhsTY.ap()[:, :], rhs_ap, start=True,
                                 stop=False)
                nc.tensor.matmul(out_ap, lhsTYl.ap()[:, :], rhs_ap, start=False,
                                 stop=True)

            mm_x1(Xp[0].ap()[0:96, 0:1], fdx1[0:104, :])
            fdy1 = pool.tile([96, 1], f16, tag="fdy1")
            nc.scalar.activation(fdy1[0:96, :], Xp[0].ap()[0:96, 0:1], AF.Relu,
                                 bias=b13col.ap()[0:96, :])
            mm_y1(Yp[0].ap()[0:67, 0:1], fdy1[0:96, :])
            fdx2 = pool.tile([104, 1], f16, tag="fdx2")
            nc.gpsimd.memset(fdx2[64:104, :], 0.0)
            nc.vector.tensor_scalar(fdx2[0:64, :], Yp[0].ap()[0:64, 0:1],
                                    byc.ap()[0:64, :], relucol.ap()[0:64, :],
                                    op.add, op.max)
            mm_x1(Xp[0].ap()[0:96, 0:1], fdx2[0:104, :])
            fdy2 = pool.tile([96, 1], f16, tag="fdy2")
            nc.scalar.activation(fdy2[0:96, :], Xp[0].ap()[0:96, 0:1], AF.Relu,
                                 bias=b13col.ap()[0:96, :])
            mm_y1(Yp[0].ap()[0:67, 0:1], fdy2[0:96, :])
            fd3 = pool.tile([3, 1], f32, tag="fd3")
            nc.scalar.activation(fd3[:, :], Yp[0].ap()[64:67, 0:1], AF.Tanh,
                                 bias=byc.ap()[64:67, :])
            nc.vector.tensor_scalar(fd3[:, :], fd3[:, :], -10.0, None, op.mult)
            fdrow = pool.tile([1, 3], f32, tag="fdrow")
            nc.gpsimd.dma_start(fdrow[0:1, 0:3], fd3[0:3, 0:1])
            nc.gpsimd.partition_broadcast(negfd.ap()[:, :], fdrow[0:1, :])

            # ================= post pass (mask, tanh-reduce, integrate) =========
            def do_post(h):
                i0, Ph = halves[h]
                nc.vector.tensor_scalar(maskS.ap()[0:Ph, :],
                                        FP[h].ap()[0:Ph, 6 * J:7 * J],
                                        1.0, None, op.is_lt)
                for d in range(3):
                    nc.vector.tensor_tensor(scrA.ap()[0:Ph, :],
                                            ZP[h].ap()[0:Ph, d * J:(d + 1) * J],
                                            maskS.ap()[0:Ph, :], op.mult)
                    nc.scalar.activation(scrB.ap()[0:Ph, :], scrA.ap()[0:Ph, :],
                                         AF.Tanh,
                                         accum_out=nf[h].ap()[0:Ph, d:d + 1])
                # nf_total = 10*sum + negfd  (negfd = -10*tanh(z4_diag))
                nc.vector.scalar_tensor_tensor(nf[h].ap()[0:Ph, :],
                                               nf[h].ap()[0:Ph, :], 10.0,
                                               negfd.ap()[0:Ph, :],
                                               op.mult, op.add)
                # ---- particle update epilogue ----
                ext = pool.tile([128, 3], f32, tag=f"ext{h}")
                nc.sync.dma_start(ext[0:Ph, :], ext_my[i0:i0 + Ph, :])
                pcol = pool.tile([128, 3], f32, tag=f"pc3{h}")
                nc.sync.dma_start(pcol[0:Ph, :], pos_my[i0:i0 + Ph, :])
                vcol = pool.tile([128, 3], f32, tag=f"vc3{h}")
                nc.sync.dma_start(vcol[0:Ph, :], vel_my[i0:i0 + Ph, :])
                mcol = pool.tile([128, 1], f32, tag=f"mm{h}")
                nc.sync.dma_start(mcol[0:Ph, :], mass_my[i0:i0 + Ph, :])
                ecol = pool.tile([128, 1], f32, tag=f"ee{h}")
                nc.sync.dma_start(ecol[0:Ph, :], ela_my[i0:i0 + Ph, :])
                fcol = pool.tile([128, 1], f32, tag=f"ff{h}")
                nc.sync.dma_start(fcol[0:Ph, :], fri_my[i0:i0 + Ph, :])

                frc = pool.tile([128, 3], f32, tag=f"frc{h}")
                nc.vector.tensor_tensor(frc[0:Ph, :], nf[h].ap()[0:Ph, :],
                                        ext[0:Ph, :], op.add)
                nc.vector.scalar_tensor_tensor(frc[0:Ph, 1:2], mcol[0:Ph, :],
                                               GRAV_Y, frc[0:Ph, 1:2],
                                               op.mult, op.add)
                imc = pool.tile([128, 1], f32, tag=f"imc{h}")
                nc.vector.reciprocal(imc[0:Ph, :], mcol[0:Ph, :])
                nc.vector.tensor_scalar(imc[0:Ph, :], imc[0:Ph, :], DT_STEP, None,
                                        op.mult)
                v1 = pool.tile([128, 3], f32, tag=f"v1{h}")
                nc.vector.scalar_tensor_tensor(v1[0:Ph, :], frc[0:Ph, :],
                                               imc[0:Ph, :], vcol[0:Ph, :],
                                               op.mult, op.add)
                sq = pool.tile([128, 3], f32, tag=f"sq{h}")
                nc.vector.tensor_tensor(sq[0:Ph, :], v1[0:Ph, :], v1[0:Ph, :],
                                        op.mult)
                spd = pool.tile([128, 1], f32, tag=f"spd{h}")
                nc.vector.reduce_sum(spd[0:Ph, :], sq[0:Ph, :], AX.X)
                nc.scalar.activation(spd[0:Ph, :], spd[0:Ph, :], AF.Sqrt)
                fm = pool.tile([128, 1], f32, tag=f"fm{h}")
                nc.vector.tensor_scalar(fm[0:Ph, :], spd[0:Ph, :], 0.1, None,
                                        op.is_gt)
                ffac = pool.tile([128, 1], f32, tag=f"ffac{h}")
                nc.vector.tensor_tensor(ffac[0:Ph, :], fcol[0:Ph, :], fm[0:Ph, :],
                                        op.mult)
                nc.vector.tensor_scalar(ffac[0:Ph, :], ffac[0:Ph, :], -DT_STEP, 1.0,
                                        op.mult, op.add)
                nc.vector.tensor_scalar(v1[0:Ph, :], v1[0:Ph, :], ffac[0:Ph, :],
                                        None, op.mult)
                p2 = pool.tile([128, 3], f32, tag=f"p2{h}")
                nc.vector.scalar_tensor_tensor(p2[0:Ph, :], v1[0:Ph, :], DT_STEP,
                                               pcol[0:Ph, :], op.mult, op.add)
                cm = pool.tile([128, 1], f32, tag=f"cm{h}")
                nc.vector.tensor_scalar(cm[0:Ph, :], p2[0:Ph, 1:2], 0.0, None,
                                        op.is_lt)
                ncm = pool.tile([128, 1], f32, tag=f"ncm{h}")
                nc.vector.tensor_scalar(ncm[0:Ph, :], cm[0:Ph, :], -1.0, 1.0,
                                        op.mult, op.add)
                nc.vector.tensor_tensor(p2[0:Ph, 1:2], p2[0:Ph, 1:2], ncm[0:Ph, :],
                                        op.mult)
                w1e = pool.tile([128, 1], f32, tag=f"w1e{h}")
                nc.vector.tensor_scalar(w1e[0:Ph, :], ecol[0:Ph, :], 1.0, None,
                                        op.add)
                nc.vector.tensor_tensor(w1e[0:Ph, :], cm[0:Ph, :], w1e[0:Ph, :],
                                        op.mult)
                nc.vector.tensor_scalar(w1e[0:Ph, :], w1e[0:Ph, :], -1.0, 1.0,
                                        op.mult, op.add)
                nc.vector.tensor_tensor(v1[0:Ph, 1:2], v1[0:Ph, 1:2], w1e[0:Ph, :],
                                        op.mult)
                nc.sync.dma_start(out_pos[i0:i0 + Ph, :], p2[0:Ph, :])
                nc.sync.dma_start(out_vel[i0:i0 + Ph, :], v1[0:Ph, :])

            # ================= main pipeline =================
            def gather(kk):
                h, p = (0, kk) if kk < 128 else (1, kk - 128)
                src = FP[h].ap()[p:p + 1, :].rearrange("p (d j) -> p d j", d=8)
                nc.gpsimd.dma_start(RX[kk % R].ap()[96:104, :], src)

            for kk in range(min(G, ni_core)):
                gather(kk)

            # lag-1 modulo schedule, chain-major: in round k, each 1024-col
            # pair P finishes row k-1 (relu / Y-matmuls / bias+relu) and then
            # starts row k (X-matmuls). This keeps next-row X work adjacent to
            # current-row Y work in the PE queue, so no engine FIFO-blocks.
            for k in range(ni_core + 2):
                rx = RX[k % R].ap()
                ry = RY[(k - 1) % 2].ap()
                rxn = RX[k % R].ap()  # tsp(k-1) writes slot ((k-1)+1)%R == k%R
                if k + G < ni_core:
                    gather(k + G)
                if 1 <= k <= ni_core + 1:
                    for s in range(S):
                        sl = slice(s * FD, (s + 1) * FD)
                        nc.scalar.activation(ry[0:96, sl], Xp[s].ap()[0:96, :],
                                             AF.Relu, bias=b13col.ap()[0:96, :])
                    for s in range(S):
                        sl = slice(s * FD, (s + 1) * FD)
                        for lhs, st in ((lhsTY, True), (lhsTYl, False)):
                            nc.tensor.matmul(Yp[s].ap()[0:67, :], lhs.ap()[:, :],
                                             ry[0:96, sl], start=st, stop=not st)
                    for s in range(S):
                        sl = slice(s * FD, (s + 1) * FD)
                        nc.vector.tensor_scalar(rxn[0:67, sl], Yp[s].ap()[0:67, :],
                                                byc.ap()[0:67, :],
                                                relucol.ap()[0:67, :],
                                                op.add, op.max)
                if k <= ni_core:
                    for s in range(S):
                        sl = slice(s * FD, (s + 1) * FD)
                        for lhs, st in ((lhsTX, True), (lhsTXl, False)):
                            nc.tensor.matmul(Xp[s].ap()[0:96, :], lhs.ap()[:, :],
                                             rx[0:104, sl], start=st, stop=not st)
                if k >= 2:
                    i = k - 2
                    h, p = (0, i) if i < 128 else (1, i - 128)
                    dst = ZP[h].ap()[p:p + 1, :].rearrange("p (d j) -> p d j", d=3)
                    src_slot = RX[(i + 2) % R].ap()
                    nc.sync.dma_start(dst, src_slot[64:67, :])
                # interleave half-1 feature generation into early rounds
                if pending and k % 8 == 5:
                    pending.pop(0)()
                if k == 133 and len(halves) > 1:
                    while pending:
                        pending.pop(0)()
                    do_post(0)
            while pending:
                pending.pop(0)()
            do_post(len(halves) - 1)

    nc.compile()
    return nc


# ----------------------------------------------------------------------------
# Host-side runner (persistent jitted executable, modeled on
# concourse.bass2jax.run_bass_via_pjrt)
# ----------------------------------------------------------------------------

def _make_runner(ni_core=NI_CORE, n_cores=NCORES):
    import jax
    import numpy as _np
    from jax.sharding import Mesh, PartitionSpec
    try:
        from jax.experimental.shard_map import shard_map
    except Exception:
        from jax.shard_map import shard_map  # newer jax
    import concourse.mybir as mybir
    from concourse import bass2jax

    nc = _build_nc(ni_core)
    bass2jax.install_neuronx_cc_hook()

    pid_name = nc.partition_id_tensor.name if nc.partition_id_tensor else None
    in_names, out_names, out_avals = [], [], []
    for alloc in nc.m.functions[0].allocations:
        if not isinstance(alloc, mybir.MemoryLocationSet):
            continue
        name = alloc.memorylocations[0].name
        if alloc.kind == "ExternalInput":
            if name != pid_name:
                in_names.append(name)
        elif alloc.kind == "ExternalOutput":
            out_names.append(name)
            out_avals.append(jax.core.ShapedArray(
                tuple(alloc.tensor_shape), mybir.dt.np(alloc.dtype)))
    n_params = len(in_names)
    all_names = in_names + out_names
    if pid_name is not None:
        all_names = all_names + [pid_name]

    def _body(*args):
        operands = list(args)
        if pid_name is not None:
            operands.append(bass2jax.partition_id_tensor())
        outs = bass2jax._bass_exec_p.bind(
            *operands,
            out_avals=tuple(out_avals),
            in_names=tuple(all_names),
            out_names=tuple(out_names),
            lowering_input_output_aliases=(),
            sim_require_finite=True,
            sim_require_nnan=True,
            nc=nc,
        )
        return tuple(outs)

    devices = jax.devices()[:n_cores]
    mesh = Mesh(_np.asarray(devices), ("core",))
    donate = tuple(range(n_params, n_params + len(out_names)))
    sharded = jax.jit(
        shard_map(_body, mesh=mesh,
                  in_specs=(PartitionSpec("core"),) * (n_params + len(out_names)),
                  out_specs=(PartitionSpec("core"),) * len(out_names),
                  check_rep=False),
        donate_argnums=donate, keep_unused=True)

    def run(in_maps):
        per_core = [[_np.asarray(m[nm]) for nm in in_names] for m in in_maps]
        concat_in = [_np.concatenate([per_core[c][i] for c in range(n_cores)], 0)
                     for i in range(n_params)]
        concat_zeros = [
            _np.zeros((n_cores * a.shape[0], *a.shape[1:]), a.dtype)
            for a in out_avals]
        out_arrs = sharded(*concat_in, *concat_zeros)
        return [
            {nm: _np.asarray(out_arrs[i]).reshape(n_cores, *out_avals[i].shape)[c]
             for i, nm in enumerate(out_names)}
            for c in range(n_cores)
        ], out_arrs

    return run, in_names


def make_in_maps(inputs, ni_core=NI_CORE, n_cores=NCORES):
    f = lambda k: np.ascontiguousarray(np.asarray(inputs[k], dtype=np.float32))
    pos, vel, mass = f("positions"), f("velocities"), f("mass")
    ext, ela, fri = f("external_forces"), f("elasticity"), f("friction")
    base = {
        "posT": np.ascontiguousarray(pos.T),
        "velT": np.ascontiguousarray(vel.T),
        "massR": mass.reshape(1, N),
        "W1": f("W1"), "b1": f("b1").reshape(1, 64),
        "W2": f("W2"), "b2": f("b2").reshape(1, 64),
        "W3": f("W3"), "b3": f("b3").reshape(1, 32),
        "W4": f("W4"), "b4": f("b4").reshape(1, 3),
    }
    maps = []
    for c in range(n_cores):
        s = slice(c * ni_core, (c + 1) * ni_core)
        maps.append(dict(
            base,
            pos_my=np.ascontiguousarray(pos[s]),
            vel_my=np.ascontiguousarray(vel[s]),
            mass_my=np.ascontiguousarray(mass[s]).reshape(ni_core, 1),
            ext_my=np.ascontiguousarray(ext[s]),
            ela_my=np.ascontiguousarray(ela[s]).reshape(ni_core, 1),
            fri_my=np.ascontiguousarray(fri[s]).reshape(ni_core, 1),
        ))
    return maps


def get_runner():
    global _RUNNER
    if _RUNNER is None:
        _RUNNER = _make_runner()
    return _RUNNER


def kernel(**inputs):
    run, _ = get_runner()
    results, _ = run(make_in_maps(inputs))
    new_pos = np.concatenate([results[c]["out_pos"] for c in range(NCORES)], 0)
    new_vel = np.concatenate([results[c]["out_vel"] for c in range(NCORES)], 0)
    return new_pos, new_vel
